# revision 1
# baseline (speedup 1.0000x reference)
"""Trainium2 Bass kernel for nn_CONV_A_64115271795341.

The module (im2col mean-centered conv + linear on window means) folds exactly
into a single 3x3 edge-padded convolution with effective weights:

  W_eff[c,k,d] = weight[c,k,d] + (w_lin[d,c] - sum_k weight[c,k,d]) / 9

Sharding: data-parallel over batch (8 images -> 8 NeuronCores), weights
replicated.

Per-core design:
  - host pre-pads each image to [64, 130*130] (edge padding), shipped fp32r.
  - SBUF xp[128, NP] fp32r: partitions 0-63 = padded image, partitions
    64-127 = same data shifted +1 element (DVE copy). A K=128 matmul at
    base offset o contracts taps j and j+1 at once: "pair" matmuls cover
    kernel taps (i,0)+(i,1) for each kernel row i.
  - taps (i,2) run as bf16 K=64 matmuls on PE column-groups 2-3
    (tile_position=(0,64)) reading a bf16 copy made by GPSIMD; fp32r
    matmuls cannot write dst partitions 64-127, bf16 can, and this keeps
    both halves of the PE array busy concurrently.
  - epilogue fused over pairs of output tiles: ACT copies PSUM-B to SBUF,
    DVE adds PSUM-A + SBUF, HWDGE stores [64, 1024] chunks.
"""

import numpy as np

C, H, W, D, B = 64, 128, 128, 64, 8
KS = 3
WP = W + 2            # 130
HP = H + 2
NP = WP * HP          # 16900 padded elems
TILE_ROWS = 4
NTILES = H // TILE_ROWS          # 32 tiles of [64, 512]
TN = TILE_ROWS * W               # 512
GROUP = 2                        # output tiles fused per epilogue op
NGROUPS = NTILES // GROUP
IN_CHUNKS = 8
DUP_CHUNKS = 4
CAST_CHUNKS = 8

_CACHE = {}


def _build(repeat=1, in_chunks=IN_CHUNKS, dup_chunks=DUP_CHUNKS,
           cast_chunks=CAST_CHUNKS, singles=True, group=GROUP,
           psum_bufs=2, out_engine="scalar", pairs=True,
           cast_engine="gpsimd", epilogue="auto", xb_host=False,
           xb_dtype="float16", allfp16=False, seq_singles=False,
           out_fp16=False):
    import concourse.bass as bass  # noqa: F401
    import concourse.mybir as mybir
    import concourse.tile as tile
    from concourse import bacc

    dt = mybir.dt
    nc = bacc.Bacc("TRN2", target_bir_lowering=False, debug=False, num_devices=8)

    if allfp16:
        return _build_allfp16(nc, mybir, tile, repeat=repeat, in_chunks=in_chunks,
                              dup_chunks=dup_chunks, group=group,
                              psum_bufs=psum_bufs, seq_singles=seq_singles,
                              out_fp16=out_fp16)
    xdt = getattr(dt, xb_dtype)
    x_d = nc.dram_tensor("x", [C, NP], dt.float32r, kind="ExternalInput")
    wpair_d = nc.dram_tensor("wpair", [128, 3 * D], dt.float32r, kind="ExternalInput")
    wsing_d = nc.dram_tensor("wsing", [C, 3 * D], xdt, kind="ExternalInput")
    if xb_host:
        xb_d = nc.dram_tensor("xb", [C, NP], xdt, kind="ExternalInput")
    out_d = nc.dram_tensor("out", [D, H * W], dt.float32, kind="ExternalOutput")

    with tile.TileContext(nc) as tc:
        with tc.tile_pool(name="io", bufs=1) as io_pool, \
             tc.tile_pool(name="outp", bufs=3) as out_pool, \
             tc.tile_pool(name="tmpp", bufs=2) as tmp_pool, \
             tc.tile_pool(name="psa", bufs=psum_bufs, space="PSUM") as psa_pool, \
             tc.tile_pool(name="psb", bufs=psum_bufs, space="PSUM") as psb_pool:

            for _rep in range(repeat):
                wpair_sb = io_pool.tile([128, 3 * D], dt.float32r,
                                        name="wpair_sb", tag="wpair_sb")
                nc.sync.dma_start(wpair_sb[:, :], wpair_d.ap()[:, :])
                wsing_sb = io_pool.tile([C, 3 * D], xdt,
                                        name="wsing_sb", tag="wsing_sb")
                nc.sync.dma_start(wsing_sb[:, :], wsing_d.ap()[:, :])

                xp = io_pool.tile([128, NP], dt.float32r, name="xp", tag="xp")
                xb = io_pool.tile([C, NP], xdt, name="xb", tag="xb")

                bnd = [NP * g // max(in_chunks, 1) for g in range(in_chunks + 1)]
                for g in range(in_chunks):
                    a, b = bnd[g], bnd[g + 1]
                    nc.sync.dma_start(xp[0:C, a:b], x_d.ap()[:, a:b])
                dbnd = [NP * g // max(dup_chunks, 1) for g in range(dup_chunks + 1)]
                for g in range(dup_chunks):
                    a, b = dbnd[g], dbnd[g + 1]
                    be = min(b, NP - 1)
                    nc.vector.tensor_copy(xp[C:128, a:be], xp[0:C, a + 1:be + 1])
                if xb_host:
                    xbnd = [NP * g // max(in_chunks, 1) for g in range(in_chunks + 1)]
                    for g in range(in_chunks):
                        a, b = xbnd[g], xbnd[g + 1]
                        nc.sync.dma_start(xb[:, a:b], xb_d.ap()[:, a:b])
                else:
                    cbnd = [NP * g // max(cast_chunks, 1) for g in range(cast_chunks + 1)]
                    cast_eng = {"gpsimd": nc.gpsimd, "vector": nc.vector,
                                "scalar": nc.scalar}[cast_engine]
                    for g in range(cast_chunks):
                        a, b = cbnd[g], cbnd[g + 1]
                        if cast_engine == "scalar":
                            nc.scalar.copy(xb[:, a:b], xp[0:C, a:b].bitcast(dt.float32))
                        else:
                            cast_eng.tensor_copy(xb[:, a:b], xp[0:C, a:b].bitcast(dt.float32))

                xv = xp.rearrange("p (r c) -> p r c", c=WP)
                xbv = xb.rearrange("p (r c) -> p r c", c=WP)

                ngroups = NTILES // group
                for grp in range(ngroups):
                    if not pairs and not singles:
                        # DMA-path-only variants: ship arbitrary bytes out
                        nc.scalar.dma_start(
                            out_d.ap()[:, group * TN * grp:group * TN * (grp + 1)],
                            xp[0:C, group * TN * grp:group * TN * (grp + 1)].bitcast(mybir.dt.float32))
                        continue
                    psA = (psa_pool.tile([64, group * TN], mybir.dt.float32,
                                         name="psA", tag="psA") if pairs else None)
                    psB = (psb_pool.tile([128, group * TN], mybir.dt.float32,
                                         name="psB", tag="psB") if singles else None)
                    for s in range(group):
                        t = grp * group + s
                        h0 = t * TILE_ROWS
                        for i in range(KS):
                            if pairs:
                                nc.tensor.matmul(
                                    psA[:, TN * s:TN * (s + 1)],
                                    lhsT=wpair_sb[:, D * i:D * (i + 1)],
                                    rhs=xv[:, h0 + i:h0 + i + TILE_ROWS, 0:W],
                                    start=(i == 0), stop=(i == KS - 1),
                                )
                            if singles:
                                nc.tensor.matmul(
                                    psB[64:128, TN * s:TN * (s + 1)],
                                    lhsT=wsing_sb[:, D * i:D * (i + 1)],
                                    rhs=xbv[:, h0 + i:h0 + i + TILE_ROWS, 2:WP],
                                    start=(i == 0), stop=(i == KS - 1),
                                    tile_position=(0, 64),
                                )
                    outt = out_pool.tile([64, group * TN], mybir.dt.float32,
                                         name="outt", tag="outt")
                    epi = epilogue
                    if epi == "auto":
                        epi = "add" if (singles and pairs) else ("copyB" if singles else "copyA")
                    if epi == "add":
                        tmp = tmp_pool.tile([64, group * TN], mybir.dt.float32,
                                            name="tmp", tag="tmp")
                        nc.scalar.copy(tmp[:, :], psB[64:128, :])
                        nc.vector.tensor_add(outt[:, :], psA[:, :], tmp[:, :])
                    elif epi == "copyB":
                        nc.vector.tensor_copy(outt[:, :], psB[64:128, :])
                    else:
                        nc.vector.tensor_copy(outt[:, :], psA[:, :])
                    dma_eng = nc.scalar if out_engine == "scalar" else nc.sync
                    dma_eng.dma_start(
                        out_d.ap()[:, group * TN * grp:group * TN * (grp + 1)],
                        outt[:, :])

    nc.compile()
    return nc


def _build_allfp16(nc, mybir, tile, repeat=1, in_chunks=8, dup_chunks=4,
                   group=GROUP, psum_bufs=2, seq_singles=False,
                   out_fp16=False):
    """All-fp16 datapath.

    Ships only the fp16 padded image (2.1MB in). Pairs (K=128, taps
    (i,0)+(i,1) via [top; top-shifted-by-1]) run on PE column-groups 0-1
    into psA[0:64]; singles (K=64, taps (i,2), top half) run concurrently
    on column-groups 2-3 into psB[64:128]. psA/psB are separate tensors so
    their banks are disjoint (no BankOverlapTracker serialization).
    Epilogue per 2-tile group: ACT copies psB->SBUF, DVE adds psA+tmp,
    one HWDGE store.
    """
    dt = mybir.dt
    xb_d = nc.dram_tensor("xb", [C, NP], dt.float16, kind="ExternalInput")
    wpair_d = nc.dram_tensor("wpair16", [128, 3 * D], dt.float16, kind="ExternalInput")
    wsing_d = nc.dram_tensor("wsing", [C, 3 * D], dt.float16, kind="ExternalInput")
    odt = dt.float16 if out_fp16 else dt.float32
    out_d = nc.dram_tensor("out", [D, H * W], odt, kind="ExternalOutput")

    sing_col = 0 if seq_singles else 64

    with tile.TileContext(nc) as tc:
        with tc.tile_pool(name="io", bufs=1) as io_pool, \
             tc.tile_pool(name="outp", bufs=3) as out_pool, \
             tc.tile_pool(name="tmpp", bufs=2) as tmp_pool, \
             tc.tile_pool(name="psa", bufs=psum_bufs, space="PSUM") as psa_pool, \
             tc.tile_pool(name="psb", bufs=psum_bufs, space="PSUM") as psb_pool:

            for _rep in range(repeat):
                wpair_sb = io_pool.tile([128, 3 * D], dt.float16,
                                        name="wpair_sb", tag="wpair_sb")
                nc.sync.dma_start(wpair_sb[:, :], wpair_d.ap()[:, :])
                wsing_sb = io_pool.tile([C, 3 * D], dt.float16,
                                        name="wsing_sb", tag="wsing_sb")
                nc.sync.dma_start(wsing_sb[:, :], wsing_d.ap()[:, :])

                xp = io_pool.tile([128, NP], dt.float16, name="xp", tag="xp")

                bnd = [NP * g // in_chunks for g in range(in_chunks + 1)]
                for g in range(in_chunks):
                    a, b = bnd[g], bnd[g + 1]
                    nc.sync.dma_start(xp[0:C, a:b], xb_d.ap()[:, a:b])
                dbnd = [NP * g // dup_chunks for g in range(dup_chunks + 1)]
                for g in range(dup_chunks):
                    a, b = dbnd[g], dbnd[g + 1]
                    be = min(b, NP - 1)
                    nc.vector.tensor_copy(xp[C:128, a:be], xp[0:C, a + 1:be + 1])

                xv = xp.rearrange("p (r c) -> p r c", c=WP)

                ngroups = NTILES // group
                for grp in range(ngroups):
                    psA = psa_pool.tile([64, group * TN], mybir.dt.float32,
                                        name="psA", tag="psA")
                    psB = psb_pool.tile([128, group * TN], mybir.dt.float32,
                                        name="psB", tag="psB")
                    for s in range(group):
                        t = grp * group + s
                        h0 = t * TILE_ROWS
                        for i in range(KS):
                            nc.tensor.matmul(
                                psA[:, TN * s:TN * (s + 1)],
                                lhsT=wpair_sb[:, D * i:D * (i + 1)],
                                rhs=xv[:, h0 + i:h0 + i + TILE_ROWS, 0:W],
                                start=(i == 0), stop=(i == KS - 1),
                            )
                            nc.tensor.matmul(
                                psB[sing_col:sing_col + 64, TN * s:TN * (s + 1)],
                                lhsT=wsing_sb[:, D * i:D * (i + 1)],
                                rhs=xv[0:C, h0 + i:h0 + i + TILE_ROWS, 2:WP],
                                start=(i == 0), stop=(i == KS - 1),
                                tile_position=(0, sing_col),
                            )
                    tmp = tmp_pool.tile([64, group * TN], mybir.dt.float32,
                                        name="tmp", tag="tmp")
                    nc.scalar.copy(tmp[:, :], psB[sing_col:sing_col + 64, :])
                    outt = out_pool.tile([64, group * TN], odt,
                                         name="outt", tag="outt")
                    nc.vector.tensor_add(outt[:, :], psA[:, :], tmp[:, :])
                    nc.scalar.dma_start(
                        out_d.ap()[:, group * TN * grp:group * TN * (grp + 1)],
                        outt[:, :])

    nc.compile()
    return nc


def _prep_inputs(x, weight, w_lin):
    import ml_dtypes
    w = weight.astype(np.float64)
    weff = w + (w_lin.astype(np.float64).T[:, None, :] - w.sum(axis=1, keepdims=True)) / 9.0
    weff = weff.astype(np.float32)                      # [C, 9, D]
    wpair = np.empty((128, 3 * D), np.float32)
    wsing = np.empty((C, 3 * D), np.float32)
    for i in range(KS):
        wpair[0:C, D * i:D * (i + 1)] = weff[:, 3 * i + 0, :]
        wpair[C:128, D * i:D * (i + 1)] = weff[:, 3 * i + 1, :]
        wsing[:, D * i:D * (i + 1)] = weff[:, 3 * i + 2, :]
    wsing = wsing.astype(np.float16)
    wpair16 = wpair.astype(np.float16)

    xp = np.pad(np.asarray(x), ((0, 0), (0, 0), (1, 1), (1, 1)), mode="edge")
    xp = xp.reshape(B, C, NP).astype(np.float32)
    xb = xp.astype(np.float16)
    return xp, wpair, wsing, xb, wpair16


OUT_FP16 = False


def kernel(x, weight, w_lin):
    from concourse.bass_utils import run_bass_kernel_spmd

    if "nc" not in _CACHE:
        _CACHE["nc"] = _build(allfp16=True, out_fp16=OUT_FP16)
    nc = _CACHE["nc"]

    xp, wpair, wsing, xb, wpair16 = _prep_inputs(x, weight, w_lin)
    in_maps = [
        {"xb": xb[b], "wpair16": wpair16, "wsing": wsing}
        for b in range(B)
    ]
    res = run_bass_kernel_spmd(nc, in_maps, core_ids=list(range(B)))
    out = np.stack([res.results[b]["out"].reshape(D, H, W) for b in range(B)])
    return out.astype(np.float32)



# revision 2
# speedup vs baseline: 1.1595x; 1.1595x over previous
"""Trainium2 Bass kernel for nn_CONV_A_64115271795341 — v4.

Same math as v3 (3 taps per matmul via [x; x>>row] contraction pairing plus
a beta column-group computing kernel-row-2 partials 2 output rows behind),
restructured for pipeline overlap:

  - psum groups of SPG=2 spans (2 banks) x 4 pool buffers = all 8 banks,
    giving 3 groups of slack between matmuls and the drain chain.
  - beta partials staged contiguously into sbB[64, H*W + 2W] (one ACT copy
    per group, partition-crossing 64:128 -> 0:64); the DVE add then reads
    a +2-row shifted window of sbB, no separate boundary ops.
  - cleanup matmuls (N=256) produce the last 2 rows' beta partials into a
    recycled psum buffer, staged into sbB's tail.
  - out[q] = psTop[q] + sbB[q+2 rows]; outt fp16, 2 big output DMAs.
"""

import numpy as np

C, H, W, D, B = 64, 128, 128, 64, 8
KS = 3
WP = W + 2            # 130
HP = H + 2
NP = WP * HP          # 16900
TILE_ROWS = 4
TN = TILE_ROWS * W    # 512
NSPANS = H // TILE_ROWS   # 32
SPG = 2                   # spans per psum group
NG = NSPANS // SPG        # 16 groups
G = SPG * TN              # 1024 cols per group
SHIFT = 2 * W             # 256: beta partials lag 2 output rows

_CACHE = {}


def _build(repeat=1, in_chunks=4, out_chunks=1, psum_bufs=4, xp_bufs=2,
           out_bufs=2, stg_dtype="float32", skip_in=False, skip_out=False,
           skip_drain=False, skip_mm=False):
    import concourse.bass as bass  # noqa: F401
    import concourse.mybir as mybir
    import concourse.tile as tile
    from concourse import bacc

    dt = mybir.dt
    sdt = getattr(dt, stg_dtype)
    nc = bacc.Bacc("TRN2", target_bir_lowering=False, debug=False, num_devices=8)

    x_d = nc.dram_tensor("x", [128, NP], dt.float16, kind="ExternalInput")
    w_d = nc.dram_tensor("w", [128, KS * 128], dt.float16, kind="ExternalInput")
    out_d = nc.dram_tensor("out", [D, H * W], dt.float16, kind="ExternalOutput")

    with tile.TileContext(nc) as tc:
        with tc.tile_pool(name="io", bufs=xp_bufs) as io_pool, \
             tc.tile_pool(name="wp", bufs=2) as w_pool, \
             tc.tile_pool(name="outp", bufs=out_bufs) as out_pool, \
             tc.tile_pool(name="stg", bufs=1) as stg_pool, \
             tc.tile_pool(name="ps", bufs=psum_bufs, space="PSUM") as ps_pool:

            for _rep in range(repeat):
                w_sb = w_pool.tile([128, KS * 128], dt.float16,
                                   name="w_sb", tag="w_sb")
                nc.sync.dma_start(w_sb[:, :], w_d.ap()[:, :])

                xp = io_pool.tile([128, NP], dt.float16, name="xp", tag="xp")
                bnd = [NP * g // in_chunks for g in range(in_chunks + 1)]
                if not skip_in:
                    for g in range(in_chunks):
                        a, b = bnd[g], bnd[g + 1]
                        nc.sync.dma_start(xp[:, a:b], x_d.ap()[:, a:b])
                else:
                    nc.sync.dma_start(xp[:, 0:NP], x_d.ap()[:, 0:NP]) if False else                     nc.sync.dma_start(xp[:, 0:64], x_d.ap()[:, 0:64])

                xv = xp.rearrange("p (r c) -> p r c", c=WP)
                outt = sbB = None
                if not skip_drain:
                    outt = out_pool.tile([D, H * W], dt.float16,
                                         name="outt", tag="outt")
                    sbB = stg_pool.tile([64, H * W + SHIFT], sdt,
                                        name="sbB", tag="sbB")

                ps_list = []
                for g in range(NG):
                    if skip_mm:
                        break
                    psP = ps_pool.tile([128, G], mybir.dt.float32,
                                       name="psP", tag="psP")
                    ps_list.append(psP)
                    for s in range(SPG if not skip_mm else 0):
                        h0 = TILE_ROWS * (SPG * g + s)
                        for j in range(KS):
                            nc.tensor.matmul(
                                psP[:, TN * s:TN * (s + 1)],
                                lhsT=w_sb[:, 128 * j:128 * (j + 1)],
                                rhs=xv[:, h0:h0 + TILE_ROWS, j:j + W],
                                start=(j == 0), stop=(j == KS - 1),
                            )
                    # stage this group's beta partials contiguously
                    if skip_drain:
                        continue
                    nc.scalar.copy(sbB[:, G * g:G * (g + 1)], psP[64:128, :])
                    if g > 0:
                        nc.vector.tensor_add(
                            outt[:, G * (g - 1):G * g],
                            ps_list[g - 1][0:64, :],
                            sbB[:, G * (g - 1) + SHIFT:G * g + SHIFT])
                # cleanup: beta partials for the last 2 output rows
                psQ = None
                if not (skip_drain or skip_mm):
                    psQ = ps_pool.tile([128, G], mybir.dt.float32,
                                       name="psP", tag="psP")
                for j in range(KS if not (skip_drain or skip_mm) else 0):
                    nc.tensor.matmul(
                        psQ[0:64, 0:SHIFT],
                        lhsT=w_sb[0:64, 128 * j + 64:128 * j + 128],
                        rhs=xv[0:64, H:H + 2, j:j + W],
                        start=(j == 0), stop=(j == KS - 1),
                    )
                if not skip_drain:
                    nc.scalar.copy(sbB[:, H * W:H * W + SHIFT],
                                   psQ[0:64, 0:SHIFT])
                    nc.vector.tensor_add(
                        outt[:, G * (NG - 1):G * NG],
                        ps_list[NG - 1][0:64, :],
                        sbB[:, G * (NG - 1) + SHIFT:G * NG + SHIFT])

                obnd = [H * W * g // out_chunks for g in range(out_chunks + 1)]
                osrc = xp[0:64, 0:H * W] if skip_drain else outt
                if not skip_out:
                    for g in range(out_chunks):
                        a, b = obnd[g], obnd[g + 1]
                        nc.scalar.dma_start(out_d.ap()[:, a:b], osrc[:, a:b])

    nc.compile()
    return nc


def _prep_inputs(x, weight, w_lin):
    w = np.asarray(weight).astype(np.float64)
    weff = w + (np.asarray(w_lin).astype(np.float64).T[:, None, :]
                - w.sum(axis=1, keepdims=True)) / 9.0
    weff = weff.astype(np.float32)                      # [C, 9, D]
    w_sb = np.zeros((128, KS * 128), np.float16)
    for j in range(KS):
        w_sb[0:C, 128 * j:128 * j + 64] = weff[:, 0 * KS + j, :]
        w_sb[C:128, 128 * j:128 * j + 64] = weff[:, 1 * KS + j, :]
        w_sb[0:C, 128 * j + 64:128 * j + 128] = weff[:, 2 * KS + j, :]

    xpad = np.pad(np.asarray(x), ((0, 0), (0, 0), (1, 1), (1, 1)), mode="edge")
    xpad = xpad.reshape(B, C, NP).astype(np.float16)
    xh = np.zeros((B, 128, NP), np.float16)
    xh[:, 0:C, :] = xpad
    xh[:, C:128, 0:NP - WP] = xpad[:, :, WP:]
    return xh, w_sb


def kernel(x, weight, w_lin):
    from concourse.bass_utils import run_bass_kernel_spmd

    if "nc" not in _CACHE:
        _CACHE["nc"] = _build()
    nc = _CACHE["nc"]

    xh, w_sb = _prep_inputs(x, weight, w_lin)
    in_maps = [{"x": xh[b], "w": w_sb} for b in range(B)]
    res = run_bass_kernel_spmd(nc, in_maps, core_ids=list(range(B)))
    out = np.stack([res.results[b]["out"].reshape(D, H, W) for b in range(B)])
    return out.astype(np.float32)
